# revision 1
# baseline (speedup 1.0000x reference)
"""Trainium2 Bass kernel for nn_ChimeraNet (encoder -> 10-step Euler RNN -> LN -> readout).

Data-parallel over 8 NeuronCores: each core gets 1024 rows of the batch and a
replicated set of (host-prefolded) weights.

Math (per core, R=1024 rows, D=1024):
    drive_in = x @ W_c + bias          with W_c = W_enc.T @ W_in (host-folded)
    h_{t+1}  = 0.8 h_t + 0.2 tanh(h_t @ W_res + drive_in),  h_0 = 0, 10 steps
    out      = inv*(h @ W2.T) + (-mu*inv)*w1 + b2           (LayerNorm folded into readout)
  where mu/var are LayerNorm stats over D, inv = rsqrt(var+eps),
  W2 = W_out * ln_g,  w1 = W2 @ 1,  b2 = W_out @ ln_b + b_out.

The integration state is stored scaled, u_t = h_t / 0.2, with 0.2 folded into
W_res and the readout weights on the host.  The recurrence becomes
    u_{t+1} = 0.8*u_t + tanh(u_t @ (0.2 W_res) + drive_in)
so the state update is a single fused scalar_tensor_tensor DVE op per tile.

On-chip layout: the state is kept TRANSPOSED (u.T, D on partitions, rows on the
free dim) so every matmul is weight-stationary (lhsT = natural weight block) and
no per-step transposes are needed.  Matmuls run as float32r (full PE rate at
N=512); accumulation is fp32 in PSUM.  LayerNorm stats over D (the partition
dim) are computed on PE: the sum via an extra ones-column in the readout lhsT,
the sum of squares via ones-vector matmuls on squared tiles.
"""

import os
import sys

import numpy as np

try:
    import concourse.bass as bass  # noqa: F401
except ImportError:  # pragma: no cover - fresh grading env without PYTHONPATH
    for p in ("/root/.axon_site", "/root/.axon_site/_ro/trn_rl_repo",
              "/root/.axon_site/_ro/pypackages", "/opt/trn_rl_repo"):
        if os.path.isdir(p) and p not in sys.path:
            sys.path.append(p)
    import concourse.bass as bass

from contextlib import ExitStack

import concourse.tile as tile
from concourse import bacc, bass_utils, mybir
from concourse.masks import make_identity

N_CORES = 8
B = 8192
R = B // N_CORES        # rows per core
D = 1024                # latent dim
KX = 784                # encoder input dim
DT_STEP = 0.2
STEPS = 10
EPS = 1e-5

F32 = mybir.dt.float32
F32R = mybir.dt.float32r
AF = mybir.ActivationFunctionType
ALU = mybir.AluOpType

KD = D // 128           # 8 k/m tiles over D
NS = R // 512           # 2 moving-dim slices of 512
KXT = [128] * 6 + [16]  # 784 = 6*128 + 16
NWARM = 6              # PE warmup matmuls (HAM un-throttle during DMA wait)


def _build_program():
    nc = bacc.Bacc("TRN2", target_bir_lowering=False, debug=False)

    x = nc.dram_tensor("x", [R, KX], F32, kind="ExternalInput").ap()
    w_c = nc.dram_tensor("w_c", [KX, D], F32, kind="ExternalInput").ap()
    w_res = nc.dram_tensor("w_res", [D, D], F32, kind="ExternalInput").ap()
    bias = nc.dram_tensor("bias", [D], F32, kind="ExternalInput").ap()
    w2a = nc.dram_tensor("w2a", [D, 11], F32, kind="ExternalInput").ap()
    w1 = nc.dram_tensor("w1", [10], F32, kind="ExternalInput").ap()
    b2 = nc.dram_tensor("b2", [10], F32, kind="ExternalInput").ap()
    out = nc.dram_tensor("out", [R, 10], F32, kind="ExternalOutput").ap()

    with tile.TileContext(nc) as tc, ExitStack() as ctx:
        state = ctx.enter_context(tc.tile_pool(name="state", bufs=1))
        consts = ctx.enter_context(tc.tile_pool(name="consts", bufs=1))
        wres_pool = ctx.enter_context(tc.tile_pool(name="wres", bufs=1))

        # persistent SBUF state: double-buffered transposed u, plus drive_in
        g = [[state.tile([128, R], F32R, name=f"g{b}_{k}", tag=f"g{b}_{k}") for k in range(KD)]
             for b in range(2)]
        drive = [state.tile([128, R], F32, name=f"dr{k}", tag=f"dr{k}") for k in range(KD)]
        wres_sb = [wres_pool.tile([128, D], F32R, name=f"wr{k}", tag=f"wr{k}") for k in range(KD)]

        with ExitStack() as mmctx:
            psum = mmctx.enter_context(
                tc.tile_pool(name="mm", bufs=4, space="PSUM"))
            if True:
                # PE warmup: dependency-free fp32 matmuls starting at t~0 pull
                # the HAM clock gate to 8/8 while the input DMAs are in flight
                # (transpose-mode does not count as PE activity for HAM).
                warm_src = consts.tile([128, 256], F32)
                nc.vector.memset(warm_src, 0.01)
                warm_sb = consts.tile([128, 1], F32)
                for w in range(NWARM):
                    wp = psum.tile([128, 512], F32, name=f"warm{w}", tag="mm")
                    nc.tensor.matmul(wp[:, :256], lhsT=warm_src[:, :128], rhs=warm_src,
                                     start=True, stop=True)
                    if w == NWARM - 1:
                        nc.vector.tensor_copy(warm_sb, wp[:, :1])  # keep-alive

                ident = consts.tile([128, 128], F32)
                make_identity(nc, ident)
                bias_sb = consts.tile([128, KD], F32)
                nc.gpsimd.dma_start(out=bias_sb, in_=bias.rearrange("(m p) -> p m", p=128))

                # ------------ encoder: x -> x.T, drive_in = x @ W_c + bias ----
                with ExitStack() as enc:
                    xn_pool = enc.enter_context(tc.tile_pool(name="xn", bufs=4))
                    xt_pool = enc.enter_context(tc.tile_pool(name="xt", bufs=1))
                    wc_pool = enc.enter_context(tc.tile_pool(name="wc", bufs=1))
                    etp = enc.enter_context(
                        tc.tile_pool(name="etp", bufs=4, space="PSUM"))

                    # x tiles on the sync (HWDGE) queue, first in program order
                    # so the transposes can start as early as possible; weights
                    # go on the gpsimd (SWDGE) queue so they don't block x.
                    xt_big = xt_pool.tile([128, len(KXT), R], F32R, name="xt_big")
                    wc_sb = [wc_pool.tile([128, D], F32R, name=f"wc{k}", tag=f"wc{k}")
                             for k in range(len(KXT))]
                    for k, kw in enumerate(KXT):
                        nc.scalar.dma_start(out=wc_sb[k][:kw, :],
                                            in_=w_c[k * 128:k * 128 + kw, :].bitcast(F32R))
                    def transpose_rt(rt):
                        xn = xn_pool.tile([128, KX], F32, name=f"xn{rt}", tag="xn")
                        nc.sync.dma_start(out=xn, in_=x[rt * 128:(rt + 1) * 128, :])
                        rsl = slice(rt * 128, (rt + 1) * 128)
                        # dependency-free matmul BEFORE the transposes: it runs
                        # during this tile's DMA wait (PE is in-order), keeping
                        # the HAM window busy so the clock gate stays at 8/8
                        wp0 = psum.tile([128, 512], F32, name=f"wmh{rt}", tag="mm")
                        nc.tensor.matmul(wp0[:, :256], lhsT=warm_src[:, :128],
                                         rhs=warm_src, start=True, stop=True)
                        # transpose two 128-blocks into one psum tile, then one
                        # strided copy moves both into the x.T tensor
                        for kp in range(3):
                            pt = etp.tile([128, 256], F32, name=f"pt{rt}_{kp}", tag="tp")
                            for h in range(2):
                                k = 2 * kp + h
                                nc.tensor.transpose(pt[:, h * 128:(h + 1) * 128],
                                                    xn[:, k * 128:(k + 1) * 128], ident)
                            src = pt.rearrange("p (two c) -> p two c", two=2)
                            dst = xt_big[:, 2 * kp:2 * kp + 2, rsl]
                            if kp % 2 == 0:
                                nc.scalar.copy(dst, src)
                            else:
                                nc.vector.tensor_copy(dst, src)
                        pt = etp.tile([128, 256], F32, name=f"pt{rt}_3", tag="tp")
                        nc.tensor.transpose(pt[:16, :128], xn[:, 768:784], ident)
                        nc.vector.tensor_copy(xt_big[:16, 6, rsl], pt[:16, :128])

                    def encoder_mms(n):
                        sl = slice(n * 512, (n + 1) * 512)
                        for m in range(KD):
                            ps = psum.tile([128, 512], F32, name=f"eps{n}_{m}", tag="mm")
                            for k, kw in enumerate(KXT):
                                nc.tensor.matmul(
                                    ps,
                                    lhsT=wc_sb[k][:kw, m * 128:(m + 1) * 128],
                                    rhs=xt_big[:kw, k, sl],
                                    start=(k == 0), stop=(k == len(KXT) - 1))
                            nc.scalar.activation(drive[m][:, sl], ps, AF.Identity,
                                                 bias=bias_sb[:, m:m + 1], scale=1.0)

                    # interleave: the n=0 encoder matmuls run (and keep the PE
                    # clock warm) while rows 4-7 are still DMA-ing in
                    for rt in range(4):
                        transpose_rt(rt)
                    encoder_mms(0)
                    for rt in range(4, 8):
                        transpose_rt(rt)
                    encoder_mms(1)

                # W_res arrives on the gpsimd queue while the encoder runs.
                for k in range(KD):
                    nc.gpsimd.dma_start(out=wres_sb[k],
                                        in_=w_res[k * 128:(k + 1) * 128, :].bitcast(F32R))

                # ------------ Euler integration loop (u-state) ----------------
                tmp = ctx.enter_context(tc.tile_pool(name="tmp", bufs=3))

                # step 0: u1 = tanh(drive_in)
                for n in range(NS):
                    sl = slice(n * 512, (n + 1) * 512)
                    for m in range(KD):
                        nc.scalar.activation(g[0][m][:, sl], drive[m][:, sl], AF.Tanh)

                sqp = ctx.enter_context(tc.tile_pool(name="sq", bufs=1))
                sq_tiles = [[sqp.tile([128, 512], F32R, name=f"sq{n}_{k}", tag=f"sq{n}_{k}")
                             for k in range(KD)] for n in range(NS)]

                for s in range(1, STEPS):
                    cur, nxt = g[(s + 1) % 2], g[s % 2]
                    for n in range(NS):
                        sl = slice(n * 512, (n + 1) * 512)
                        for m in range(KD):
                            ps = psum.tile([128, 512], F32, name=f"ps{s}_{n}_{m}", tag="mm")
                            for k in range(KD):
                                nc.tensor.matmul(
                                    ps,
                                    lhsT=wres_sb[k][:, m * 128:(m + 1) * 128],
                                    rhs=cur[k][:, sl],
                                    start=(k == 0), stop=(k == KD - 1))
                            d = tmp.tile([128, 512], F32, name=f"d{s}_{n}_{m}", tag="d")
                            nc.vector.tensor_add(d, ps, drive[m][:, sl])
                            t = tmp.tile([128, 512], F32, name=f"t{s}_{n}_{m}", tag="t")
                            nc.scalar.activation(t, d, AF.Tanh)
                            # u' = 0.8*u + t  (single fused op)
                            nc.vector.scalar_tensor_tensor(
                                nxt[m][:, sl], in0=cur[m][:, sl], scalar=1.0 - DT_STEP,
                                in1=t, op0=ALU.mult, op1=ALU.add)
                            if s == STEPS - 1:
                                # square for the LN variance, overlapped here so
                                # the tail matmuls don't wait on a serial chain
                                nc.scalar.activation(sq_tiles[n][m], nxt[m][:, sl],
                                                     AF.Square)

                gfin = g[(STEPS - 1) % 2]

                # ------------ tail: LN stats + readout (matmul part) ----------
                tail = ctx.enter_context(tc.tile_pool(name="tail", bufs=1))

                ones_f32 = tail.tile([128, 1], F32)
                nc.vector.memset(ones_f32, 1.0)
                ones_sb = tail.tile([128, 1], F32R)
                nc.scalar.copy(ones_sb, ones_f32)
                eps_sb = tail.tile([128, 1], F32)
                nc.vector.memset(eps_sb, EPS)
                # w2a = [0.2*W2.T | ones] : readout weights + S1 column
                w2a_sb = tail.tile([128, KD, 11], F32R)
                nc.gpsimd.dma_start(out=w2a_sb,
                                    in_=w2a.rearrange("(k p) o -> p k o", p=128).bitcast(F32R))
                w1_bc = tail.tile([128, 10], F32)
                nc.gpsimd.dma_start(out=w1_bc, in_=bass.AP(tensor=w1.tensor, offset=w1.offset,
                                                           ap=[[0, 128]] + list(w1.ap)))
                b2_bc = tail.tile([128, 10], F32)
                nc.gpsimd.dma_start(out=b2_bc, in_=bass.AP(tensor=b2.tensor, offset=b2.offset,
                                                           ap=[[0, 128]] + list(b2.ap)))

                s2_sb = tail.tile([1, R], F32)
                y_sb = tail.tile([11, R], F32)

                # per-n readout matmuls, with the per-row-tile stat/combine
                # chains interleaved so the n=0 half finishes while n=1 runs.
                # y_sb rows 0-9 = 0.2*(W2 @ u.T) = W2 @ h.T;  row 10 = sum_D u.
                tp2ctx = ExitStack()
                tp2 = tp2ctx.enter_context(
                    tc.tile_pool(name="tp2", bufs=4, space="PSUM"))
                for n in range(NS):
                    sl = slice(n * 512, (n + 1) * 512)
                    yp = psum.tile([11, 512], F32, name=f"yp{n}", tag="mm")
                    for k in range(KD):
                        nc.tensor.matmul(yp, lhsT=w2a_sb[:, k, :],
                                         rhs=gfin[k][:, sl],
                                         start=(k == 0), stop=(k == KD - 1))
                    nc.scalar.copy(y_sb[:, sl], yp)
                    s2 = psum.tile([1, 512], F32, name=f"s2p{n}", tag="mm")
                    for k in range(KD):
                        nc.tensor.matmul(s2, lhsT=ones_sb, rhs=sq_tiles[n][k],
                                         start=(k == 0), stop=(k == KD - 1))
                    nc.scalar.copy(s2_sb[:, sl], s2)

                    for rt in range(n * 4, (n + 1) * 4):
                        sl = slice(rt * 128, (rt + 1) * 128)
                        yn = tp2.tile([128, 11], F32, name=f"yn{rt}", tag="st")
                        nc.tensor.transpose(yn, y_sb[:, sl], ident[:11, :11])
                        p2 = tp2.tile([128, 1], F32, name=f"p2_{rt}", tag="st")
                        nc.tensor.transpose(p2, s2_sb[:, sl], ident[:1, :1])
                        mu_n = tail.tile([128, 1], F32, name=f"mu{rt}", tag="mu", bufs=2)
                        nc.scalar.mul(mu_n, yn[:, 10:11], -DT_STEP / D)   # -mean(h)
                        ex2 = tail.tile([128, 1], F32, name=f"ex2_{rt}", tag="ex2", bufs=2)
                        nc.scalar.mul(ex2, p2, DT_STEP * DT_STEP / D)     # E[h^2]
                        var = tail.tile([128, 1], F32, name=f"var{rt}", tag="var", bufs=2)
                        # var = E[h^2] - mu^2 = -(mu_n*mu_n) + ex2
                        nc.vector.scalar_tensor_tensor(var, in0=mu_n, scalar=-1.0,
                                                       op0=ALU.mult, in1=mu_n,
                                                       op1=ALU.mult)
                        nc.vector.tensor_add(var, var, ex2)
                        sd = tail.tile([128, 1], F32, name=f"sd{rt}", tag="sd", bufs=2)
                        nc.scalar.activation(sd, var, AF.Sqrt, bias=eps_sb, scale=1.0)
                        inv = tail.tile([128, 1], F32, name=f"inv{rt}", tag="inv", bufs=2)
                        nc.vector.reciprocal(inv, sd)
                        qn = tail.tile([128, 1], F32, name=f"qn{rt}", tag="qn", bufs=2)
                        nc.vector.tensor_mul(qn, mu_n, inv)               # -mu*inv

                        t1 = tail.tile([128, 10], F32, name=f"t1_{rt}", tag="t1", bufs=2)
                        nc.vector.tensor_scalar_mul(t1, yn[:, 0:10], inv)
                        t2 = tail.tile([128, 10], F32, name=f"t2_{rt}", tag="t2", bufs=2)
                        nc.vector.scalar_tensor_tensor(t2, in0=w1_bc, scalar=qn,
                                                       in1=t1, op0=ALU.mult, op1=ALU.add)
                        o = tail.tile([128, 10], F32, name=f"o{rt}", tag="o", bufs=2)
                        nc.vector.tensor_add(o, t2, b2_bc)
                        nc.sync.dma_start(out=out[sl, :], in_=o)
                tp2ctx.close()

    nc.compile()
    return nc


_NC_CACHE = None


def _get_program():
    global _NC_CACHE
    if _NC_CACHE is None:
        _NC_CACHE = _build_program()
    return _NC_CACHE


def _prepare_in_maps(inputs):
    x = np.asarray(inputs["x"], dtype=np.float32)
    w_enc = np.asarray(inputs["W_enc"], dtype=np.float32)
    w_res = np.asarray(inputs["W_res"], dtype=np.float32)
    w_in = np.asarray(inputs["W_in"], dtype=np.float32)
    bias = np.asarray(inputs["bias"], dtype=np.float32)
    ln_g = np.asarray(inputs["ln_g"], dtype=np.float32)
    ln_b = np.asarray(inputs["ln_b"], dtype=np.float32)
    w_out = np.asarray(inputs["W_out"], dtype=np.float32)
    b_out = np.asarray(inputs["b_out"], dtype=np.float32)

    w_c = (w_enc.T.astype(np.float64) @ w_in.astype(np.float64)).astype(np.float32)
    w2 = w_out * ln_g[None, :]                       # [10, D]
    # state is u = h/0.2: fold 0.2 into W_res (matmul input) and readout/stats
    w_res_s = (DT_STEP * w_res.astype(np.float64)).astype(np.float32)
    w2a = np.empty((D, 11), np.float32)
    w2a[:, :10] = DT_STEP * w2.T                     # readout: gives W2 @ h.T
    w2a[:, 10] = 1.0                                 # S1 column: sum_D u
    w1v = w2.sum(axis=1).astype(np.float32)
    b2v = (w_out.astype(np.float64) @ ln_b.astype(np.float64)
           + b_out.astype(np.float64)).astype(np.float32)

    shared = {
        "w_c": np.ascontiguousarray(w_c),
        "w_res": np.ascontiguousarray(w_res_s),
        "bias": np.ascontiguousarray(bias),
        "w2a": np.ascontiguousarray(w2a),
        "w1": np.ascontiguousarray(w1v),
        "b2": np.ascontiguousarray(b2v),
    }
    in_maps = []
    for c in range(N_CORES):
        m = dict(shared)
        m["x"] = np.ascontiguousarray(x[c * R:(c + 1) * R, :])
        in_maps.append(m)
    return in_maps


def run(inputs, trace=False, tmpdir=None):
    """Run on 8 NeuronCores; returns (out [8192,10], BassKernelResults)."""
    nc = _get_program()
    in_maps = _prepare_in_maps(inputs)
    res = bass_utils.run_bass_kernel_spmd(
        nc, in_maps, core_ids=list(range(N_CORES)), trace=trace, tmpdir=tmpdir)
    outs = [np.asarray(r["out"]) for r in res.results]
    return np.concatenate(outs, axis=0), res


def kernel(**inputs):
    out, _ = run(inputs, trace=False)
    return out

